# revision 11
# baseline (speedup 1.0000x reference)
"""Causal self-attention (GQA + RoPE + QK-RMSNorm) Trainium2 Bass kernel.

Sharding (8 cores): core c -> batch b = c//4, kv-head j = c%4, q-heads 4j..4j+3.
Each core computes its 4 heads' attention for its batch plus the partial
output projection against wo[:, 512j:512j+512]; the host sums the 4 partials
per batch.

v2: single chunk-pipelined pass. Per 512-token chunk ci:
  proj(K,Q0..Q3,V over xc[ci]) -> rope-core + rms-Ln per target ->
  V-transpose -> finals (Exp+mul -> ktb/qtb chunk) -> wo(ci-1) -> attention(ci).
This keeps the tensor engine fed across phase boundaries: the next chunk's
projections fill attention's exp/recip dependency bubbles, and wo is emitted
one chunk late so its yt inputs are long since ready.

All host-side tensors are packed so every DMA is (128, N) contiguous per
partition (4-16KB descriptors). fp16 operands everywhere (same PE rate as
bf16, 8x less quantization noise); attention exp is exp(s-2) to keep the
row sums comfortably inside fp16 range (the 2 cancels in the softmax ratio).
"""

import math

import numpy as np

B, T, D = 2, 2048, 2048
N_HEAD, N_KV_HEAD = 16, 4
HD = 128
HPC = N_HEAD // N_KV_HEAD  # q heads per core = 4
N_CORES = 8
ROPE_THETA = 10000.0
EPS = float(np.finfo(np.float32).eps)
NEG = -1.0e5
ESHIFT = -2.0  # attention exp computes exp(s + ESHIFT); cancels in av/sums

KT = D // 128  # 16 contraction k-tiles
NCH = T // 512  # 4 Tq chunks


# --------------------------------------------------------------------------
# host-side constant tables
# --------------------------------------------------------------------------

def round_fp32r(a: np.ndarray) -> np.ndarray:
    """Round fp32 to the fp32r grid (11-bit mantissa, round-to-nearest-even)."""
    b = np.ascontiguousarray(a, dtype=np.float32).view(np.uint32)
    r = (b + np.uint32(0x7FF) + ((b >> np.uint32(12)) & np.uint32(1))) & np.uint32(0xFFFFF000)
    return r.view(np.float32)


def _f16(a: np.ndarray):
    return np.ascontiguousarray(a).astype(np.float16)


def _perm128() -> np.ndarray:
    # evens then odds within one head's 128 dims
    return np.concatenate([np.arange(0, HD, 2), np.arange(1, HD, 2)])


def _rope_tables(t: int, norm_w: np.ndarray) -> tuple[np.ndarray, np.ndarray]:
    """A, B tables (128, t) for rope in permuted-QT layout, norm weight
    folded in: newQT = QT * A + SWAP64(QT) * B."""
    inv_freq = (1.0 / (ROPE_THETA ** (np.arange(0, HD, 2).astype(np.float32) / HD))).astype(np.float32)
    ang = np.arange(t, dtype=np.float32)[:, None] * inv_freq[None, :]  # (t, 64)
    cos = np.cos(ang).T.astype(np.float32)  # (64, t)
    sin = np.sin(ang).T.astype(np.float32)
    w = norm_w[_perm128()].astype(np.float32)  # (128,)
    a = np.concatenate([cos, cos], axis=0) * w[:, None]
    b = np.concatenate([-sin, sin], axis=0) * w[:, None]
    return np.ascontiguousarray(a), np.ascontiguousarray(b)


def _swap64() -> np.ndarray:
    # lhsT for out = SWAP64(rhs): lhsT[k, p] = 1 iff k == (p + 64) % 128
    p = np.arange(128)
    m = np.zeros((128, 128), dtype=np.float32)
    m[(p + 64) % 128, p] = 1.0
    return m


def _tri() -> np.ndarray:
    # scores^T diagonal-block mask: rows kk (key), cols qq (query), valid kk<=qq
    kk = np.arange(128)[:, None]
    qq = np.arange(128)[None, :]
    return np.where(kk <= qq, 0.0, NEG).astype(np.float32)


def _pack_w(w_rows: np.ndarray) -> np.ndarray:
    """(128, D) weight rows -> (128, KT*128) fp16, wt[p, 128k+i] = W[i, 128k+p]."""
    return _f16(w_rows.T.reshape(KT, 128, 128).transpose(1, 0, 2).reshape(128, KT * 128))


def _pack_x_chunk(xT: np.ndarray, ci: int) -> np.ndarray:
    """xT (D, T) slice -> (128, KT*512) fp16, xc[p, 512k+j] = xT[128k+p, 512ci+j]."""
    s = xT[:, 512 * ci : 512 * (ci + 1)]
    return _f16(s.reshape(KT, 128, 512).transpose(1, 0, 2).reshape(128, KT * 512))


# --------------------------------------------------------------------------
# device program
# --------------------------------------------------------------------------

def build_program(t: int):
    import concourse.bass as bass  # noqa: F401
    import concourse.tile as tile
    from concourse import bacc, mybir

    f32 = mybir.dt.float32
    f32r = mybir.dt.float32r
    f16 = mybir.dt.float16

    nch = t // 512

    nc = bacc.Bacc("TRN2", target_bir_lowering=False, debug=False, num_devices=N_CORES)

    io = {"t": t, "nch": nch}
    io["wtk_d"] = nc.dram_tensor("wtk", [128, KT * 128], f16, kind="ExternalInput").ap()
    for m in range(HPC):
        io[f"wtq{m}_d"] = nc.dram_tensor(f"wtq{m}", [128, KT * 128], f16, kind="ExternalInput").ap()
    io["wtv_d"] = nc.dram_tensor("wtv", [128, KT * 128], f16, kind="ExternalInput").ap()
    for ci in range(nch):
        io[f"xc{ci}_d"] = nc.dram_tensor(f"xc{ci}", [128, KT * 512], f16, kind="ExternalInput").ap()
    for h in range(HPC):
        io[f"wo{h}_d"] = nc.dram_tensor(f"wo{h}", [128, D], f16, kind="ExternalInput").ap()
    for nm in ("aq", "bq", "ak", "bk"):
        io[f"{nm}_d"] = nc.dram_tensor(nm, [128, t], f16, kind="ExternalInput").ap()
    io["p64_d"] = nc.dram_tensor("p64", [128, 128], f32r, kind="ExternalInput").ap()
    io["tri_d"] = nc.dram_tensor("tri", [128, 128], f32, kind="ExternalInput").ap()
    io["ones_d"] = nc.dram_tensor("ones", [128, 128], f16, kind="ExternalInput").ap()
    io["ident_d"] = nc.dram_tensor("ident", [128, 128], f16, kind="ExternalInput").ap()
    io["out_d"] = nc.dram_tensor("out_partial", [t // 128, 128, D], f16, kind="ExternalOutput").ap()

    with tile.TileContext(nc) as tc:
        _build_tile(tc, io)

    nc.compile()
    return nc


def _build_tile(tc, io):
    from concourse import mybir

    nc = tc.nc
    f32 = mybir.dt.float32
    f32r = mybir.dt.float32r
    f16 = mybir.dt.float16
    AF = mybir.ActivationFunctionType

    t = io["t"]
    nch = io["nch"]
    out_d = io["out_d"]

    with (
        tc.tile_pool(name="persist", bufs=1) as pp,
        tc.tile_pool(name="xcp", bufs=3) as xcp,
        tc.tile_pool(name="ps_proj", bufs=2, space="PSUM") as ps_proj,
        tc.tile_pool(name="ps_w", bufs=3, space="PSUM") as ps_w,
        tc.tile_pool(name="ps_av", bufs=2, space="PSUM") as ps_av,
        tc.tile_pool(name="ps_sum", bufs=1, space="PSUM") as ps_sum,
    ):
        # ---- persistent sbuf ----
        qtb = [pp.tile([128, t], f16, tag=f"qtb{h}", name=f"qtb{h}") for h in range(HPC)]
        ktb = pp.tile([128, t], f16, tag="ktb", name="ktb")
        vb = pp.tile([128, t], f16, tag="vb", name="vb")
        wt = {}
        wt["k"] = pp.tile([128, KT * 128], f16, tag="wtk", name="wtk")
        for m in range(HPC):
            wt[f"q{m}"] = pp.tile([128, KT * 128], f16, tag=f"wtq{m}", name=f"wtq{m}")
        wt["v"] = pp.tile([128, KT * 128], f16, tag="wtv", name="wtv")
        wo_t = [pp.tile([128, D], f16, tag=f"wo{h}", name=f"wo{h}") for h in range(HPC)]
        a_q = pp.tile([128, t], f16, tag="a_q", name="a_q")
        b_q = pp.tile([128, t], f16, tag="b_q", name="b_q")
        a_k = pp.tile([128, t], f16, tag="a_k", name="a_k")
        b_k = pp.tile([128, t], f16, tag="b_k", name="b_k")
        p64 = pp.tile([128, 128], f32r, tag="p64", name="p64")
        tri = pp.tile([128, 128], f32, tag="tri", name="tri")
        ones = pp.tile([128, 128], f16, tag="ones", name="ones")
        ident = pp.tile([128, 128], f16, tag="ident", name="ident")
        c_eps = pp.tile([128, 1], f32, tag="c_eps", name="c_eps")
        c_rkb = pp.tile([128, 1], f32, tag="c_rkb", name="c_rkb")
        c_zero = pp.tile([128, 1], f32, tag="c_zero", name="c_zero")
        c_shift = pp.tile([128, 1], f32, tag="c_shift", name="c_shift")
        nc.gpsimd.memset(c_eps[:], EPS)
        nc.gpsimd.memset(c_rkb[:], -0.5 * math.log(float(HD)))
        nc.gpsimd.memset(c_zero[:], 0.0)
        nc.gpsimd.memset(c_shift[:], ESHIFT)

        # ---- input DMA, in consumption order ----
        # weights + x chunks on the sync (SP/HWDGE) queue; tables + consts on
        # the gpsimd (SWDGE) queue so the two streams interleave.
        nc.sync.dma_start(wt["k"][:], io["wtk_d"])
        xc = {}
        for ci in range(3):
            xc[ci] = xcp.tile([128, KT * 512], f16, tag="xc", name=f"xc{ci}")
        nc.sync.dma_start(xc[0][:], io["xc0_d"])
        nc.sync.dma_start(wt["q0"][:], io["wtq0_d"])
        nc.gpsimd.dma_start(p64[:], io["p64_d"])
        nc.gpsimd.dma_start(tri[:], io["tri_d"])
        nc.gpsimd.dma_start(ones[:], io["ones_d"])
        nc.gpsimd.dma_start(ident[:], io["ident_d"])
        nc.gpsimd.dma_start(a_k[:], io["ak_d"])
        nc.gpsimd.dma_start(b_k[:], io["bk_d"])
        nc.sync.dma_start(wt["q1"][:], io["wtq1_d"])
        nc.gpsimd.dma_start(a_q[:], io["aq_d"])
        nc.gpsimd.dma_start(b_q[:], io["bq_d"])
        nc.sync.dma_start(wt["q2"][:], io["wtq2_d"])
        nc.sync.dma_start(wt["q3"][:], io["wtq3_d"])
        nc.sync.dma_start(wt["v"][:], io["wtv_d"])
        nc.sync.dma_start(xc[1][:], io["xc1_d"])
        for h in range(HPC):
            nc.sync.dma_start(wo_t[h][:], io[f"wo{h}_d"])
        nc.sync.dma_start(xc[2][:], io["xc2_d"])
        # xc3 is DMA'd after chunk 0's projections free the xc buffer slot

        with (
            tc.tile_pool(name="rawp", bufs=3) as rawp,
            tc.tile_pool(name="sqp", bufs=2) as sqp,
            tc.tile_pool(name="scrp", bufs=3) as scrp,
            tc.tile_pool(name="lnp", bufs=6) as lnp,
            tc.tile_pool(name="q1p", bufs=6) as q1p,
            tc.tile_pool(name="rtp", bufs=2) as rtp,
            tc.tile_pool(name="vtp", bufs=2) as vtp,
            tc.tile_pool(name="exp", bufs=12) as expool,
            tc.tile_pool(name="rsp", bufs=4) as rspool,
            tc.tile_pool(name="obp", bufs=2) as obp,
        ):
            targets = ["k", "q0", "q1", "q2", "q3", "v"]

            def emit_wo_slice(ci, mi):
                m = 4 * ci + mi
                ob = obp.tile([128, D], f16, tag="ob", name="ob")
                for n in range(D // 512):
                    wops = ps_proj.tile([128, 512], f32, tag="proj", name="wo_ps")
                    for h in range(HPC):
                        nc.tensor.matmul(
                            wops[:],
                            yt[h][:, 128 * m : 128 * (m + 1)],
                            wo_t[h][:, 512 * n : 512 * (n + 1)],
                            start=(h == 0),
                            stop=(h == HPC - 1),
                        )
                    if (m + n) % 2 == 0:
                        nc.scalar.copy(ob[:, 512 * n : 512 * (n + 1)], wops[:])
                    else:
                        nc.vector.tensor_copy(ob[:, 512 * n : 512 * (n + 1)], wops[:])
                nc.sync.dma_start(out_d[m], ob[:])

            yt = [pp.tile([128, t], f16, tag=f"yt{h}", name=f"yt{h}") for h in range(HPC)]

            for ci in range(nch):
                sl = slice(512 * ci, 512 * (ci + 1))
                # ---- projections + rope-core/rms-Ln per target ----
                lnts = {}
                q1bs = {}
                vt = None
                for tgt in targets:
                    ps = ps_proj.tile([128, 512], f32, tag="proj", name="proj_ps")
                    for k in range(KT):
                        nc.tensor.matmul(
                            ps[:],
                            wt[tgt][:, 128 * k : 128 * (k + 1)],
                            xc[ci][:, 512 * k : 512 * (k + 1)],
                            start=(k == 0),
                            stop=(k == KT - 1),
                        )
                    if tgt == "v":
                        vt = vtp.tile([128, 512], f16, tag="vt", name="vt")
                        nc.vector.tensor_copy(vt[:], ps[:])
                        continue
                    raw = rawp.tile([128, 512], f32r, tag="raw", name="raw")
                    nc.scalar.copy(raw[:], ps[:])
                    # rms sum-of-squares -> Ln
                    sq = sqp.tile([128, 512], f16, tag="sq", name="sq")
                    nc.gpsimd.tensor_mul(sq[:], raw[:], raw[:])
                    ssq = ps_w.tile([128, 512], f32, tag="w", name="ssq_ps")
                    nc.tensor.matmul(ssq[:], ones[:], sq[:])
                    lnt = lnp.tile([128, 512], f16, tag="lnt", name="lnt")
                    nc.scalar.activation(lnt[:], ssq[:], AF.Ln, bias=c_eps[:], scale=1.0 / HD)
                    lnts[tgt] = lnt
                    # rope core
                    swp = ps_w.tile([128, 512], f32, tag="w", name="swp_ps")
                    nc.tensor.matmul(swp[:], p64[:], raw[:])
                    atab, btab = (a_k, b_k) if tgt == "k" else (a_q, b_q)
                    q1 = scrp.tile([128, 512], f32, tag="scr", name="q1")
                    nc.vector.tensor_mul(q1[:], raw[:], atab[:, sl])
                    m2 = scrp.tile([128, 512], f32, tag="scr", name="m2")
                    nc.vector.tensor_mul(m2[:], swp[:], btab[:, sl])
                    q1b = q1p.tile([128, 512], f16, tag="q1b", name="q1b")
                    nc.vector.tensor_add(q1b[:], q1[:], m2[:])
                    q1bs[tgt] = q1b

                # xc3 load goes in the slot xc0 frees up
                if ci == 0 and nch > 3:
                    xc[3] = xcp.tile([128, KT * 512], f16, tag="xc", name="xc3")
                    nc.sync.dma_start(xc[3][:], io["xc3_d"])

                # ---- V transpose: (hd, Tk) -> (Tk, hd) blocks ----
                for c4 in range(4):
                    vps = ps_w.tile([128, 128], f16, tag="w", name="vtr_ps")
                    nc.tensor.transpose(vps[:], vt[:, 128 * c4 : 128 * (c4 + 1)], ident[:])
                    nc.vector.tensor_copy(vb[:, 512 * ci + 128 * c4 : 512 * ci + 128 * (c4 + 1)], vps[:])

                # ---- finals: r = exp(-0.5*ln(mean+eps)) [+ head-dim scale for k] ----
                for tgt in ["k", "q0", "q1", "q2", "q3"]:
                    dstb = ktb if tgt == "k" else qtb[int(tgt[1])]
                    bias = c_rkb if tgt == "k" else c_zero
                    r_t = rtp.tile([128, 512], f32, tag="r_t", name="r_t")
                    nc.scalar.activation(r_t[:], lnts[tgt][:], AF.Exp, bias=bias[:], scale=-0.5)
                    nc.vector.tensor_mul(dstb[:, sl], q1bs[tgt][:], r_t[:])

                # ---- attention for chunk ci; wo m-slices of the previous
                # chunk (inputs long since ready) interleave after each head ----
                for h in range(HPC):
                    av = ps_av.tile([128, 512], f32, tag="av", name="av_ps")
                    sums = ps_sum.tile([128, 512], f32, tag="sums", name="sums_ps")
                    nb = 4 * ci + 4
                    for c in range(nb):
                        diag = c >= 4 * ci
                        r = c - 4 * ci if diag else 0
                        w0 = 128 * r
                        sc = ps_w.tile([128, 512], f32, tag="w", name="sc_ps")
                        nc.tensor.matmul(
                            sc[:, w0:512],
                            ktb[:, 128 * c : 128 * (c + 1)],
                            qtb[h][:, 512 * ci + w0 : 512 * (ci + 1)],
                        )
                        if diag:
                            nc.vector.tensor_add(sc[:, w0 : w0 + 128], sc[:, w0 : w0 + 128], tri[:])
                        ex = expool.tile([128, 512], f16, tag="ex", name="ex")
                        nc.scalar.activation(ex[:, w0:512], sc[:, w0:512], AF.Exp, bias=c_shift[:])
                        nc.tensor.matmul(
                            sums[:, w0:512],
                            ones[:],
                            ex[:, w0:512],
                            start=(c == 0),
                            stop=(c == nb - 1),
                        )
                        nc.tensor.matmul(
                            av[:, w0:512],
                            vb[:, 128 * c : 128 * (c + 1)],
                            ex[:, w0:512],
                            start=(c == 0),
                            stop=(c == nb - 1),
                        )
                    rs = rspool.tile([128, 512], f32, tag="rs", name="rs")
                    rs2 = rspool.tile([128, 512], f32, tag="rs", name="rs2")
                    nc.vector.reciprocal_approx_accurate(rs[:], sums[:], rs2[:])
                    nc.vector.tensor_mul(yt[h][:, sl], av[:], rs[:])
                    if ci > 0:
                        emit_wo_slice(ci - 1, h)

            for mi in range(4):
                emit_wo_slice(nch - 1, mi)


# --------------------------------------------------------------------------
# host wrapper
# --------------------------------------------------------------------------

_PROGRAM_CACHE: dict[int, object] = {}
TRACE = False


def _get_program(t: int):
    if t not in _PROGRAM_CACHE:
        _PROGRAM_CACHE[t] = build_program(t)
    return _PROGRAM_CACHE[t]


def make_core_inputs(x, wq, wk, wv, wo, q_norm_w, k_norm_w, t: int):
    """Build the 8 per-core input dicts (numpy, host-side sharding)."""
    perm = _perm128()
    aq, bq = _rope_tables(t, q_norm_w)
    ak, bk = _rope_tables(t, k_norm_w)
    aq, bq, ak, bk = (v.astype(np.float16) for v in (aq, bq, ak, bk))
    p64 = round_fp32r(_swap64())
    tri = _tri()
    ones = np.ones((128, 128), dtype=np.float16)
    ident = np.eye(128, dtype=np.float32).astype(np.float16)

    nch = t // 512
    xcs = {}  # (b, ci) -> packed chunk
    for b in range(B):
        xT = np.ascontiguousarray(x[b].T)
        for ci in range(nch):
            xcs[(b, ci)] = _pack_x_chunk(xT, ci)

    in_maps = []
    for core in range(N_CORES):
        b = core // N_KV_HEAD
        j = core % N_KV_HEAD
        m = {}
        krows = 128 * j + perm
        m["wtk"] = _pack_w(wk[krows, :])
        for hh in range(HPC):
            qrows = 128 * (HPC * j + hh) + perm
            m[f"wtq{hh}"] = _pack_w(wq[qrows, :])
        m["wtv"] = _pack_w(wv[128 * j : 128 * (j + 1), :])
        for ci in range(nch):
            m[f"xc{ci}"] = xcs[(b, ci)]
        for h in range(HPC):
            m[f"wo{h}"] = _f16(wo[:, 512 * j + 128 * h : 512 * j + 128 * (h + 1)].T)
        m.update({"aq": aq, "bq": bq, "ak": ak, "bk": bk, "p64": p64, "tri": tri,
                  "ones": ones, "ident": ident})
        in_maps.append(m)
    return in_maps


def kernel(x, wq, wk, wv, wo, q_norm_w, k_norm_w):
    x = np.asarray(x, dtype=np.float32)
    wq = np.asarray(wq, dtype=np.float32)
    wk = np.asarray(wk, dtype=np.float32)
    wv = np.asarray(wv, dtype=np.float32)
    wo = np.asarray(wo, dtype=np.float32)
    q_norm_w = np.asarray(q_norm_w, dtype=np.float32)
    k_norm_w = np.asarray(k_norm_w, dtype=np.float32)

    t = x.shape[1]
    nc = _get_program(t)
    in_maps = make_core_inputs(x, wq, wk, wv, wo, q_norm_w, k_norm_w, t)

    from concourse import bass_utils

    res = bass_utils.run_bass_kernel_spmd(
        nc,
        in_maps,
        core_ids=list(range(N_CORES)),
        trace=TRACE,
        trace_cores=[0] if TRACE else None,
    )
    kernel.last_results = res

    out = np.zeros((B, t, D), dtype=np.float32)
    for core in range(N_CORES):
        b = core // N_KV_HEAD
        out[b] += res.results[core]["out_partial"].reshape(t, D).astype(np.float32)
    return out


kernel.last_results = None


# revision 14
# speedup vs baseline: 1.0791x; 1.0791x over previous
"""Causal self-attention (GQA + RoPE + QK-RMSNorm) Trainium2 Bass kernel.

Sharding (8 cores): core c -> batch b = c//4, kv-head j = c%4, q-heads 4j..4j+3.
Each core computes its 4 heads' attention for its batch plus the partial
output projection against wo[:, 512j:512j+512]; the host sums the 4 partials
per batch.

v2: single chunk-pipelined pass. Per 512-token chunk ci:
  proj(K,Q0..Q3,V over xc[ci]) -> rope-core + rms-Ln per target ->
  V-transpose -> finals (Exp+mul -> ktb/qtb chunk) -> wo(ci-1) -> attention(ci).
This keeps the tensor engine fed across phase boundaries: the next chunk's
projections fill attention's exp/recip dependency bubbles, and wo is emitted
one chunk late so its yt inputs are long since ready.

All host-side tensors are packed so every DMA is (128, N) contiguous per
partition (4-16KB descriptors). fp16 operands everywhere (same PE rate as
bf16, 8x less quantization noise); attention exp is exp(s-2) to keep the
row sums comfortably inside fp16 range (the 2 cancels in the softmax ratio).
"""

import math

import numpy as np

B, T, D = 2, 2048, 2048
N_HEAD, N_KV_HEAD = 16, 4
HD = 128
HPC = N_HEAD // N_KV_HEAD  # q heads per core = 4
N_CORES = 8
ROPE_THETA = 10000.0
EPS = float(np.finfo(np.float32).eps)
NEG = -1.0e5
ESHIFT = -2.0  # attention exp computes exp(s + ESHIFT); cancels in av/sums

KT = D // 128  # 16 contraction k-tiles
NCH = T // 512  # 4 Tq chunks


# --------------------------------------------------------------------------
# host-side constant tables
# --------------------------------------------------------------------------

def round_fp32r(a: np.ndarray) -> np.ndarray:
    """Round fp32 to the fp32r grid (11-bit mantissa, round-to-nearest-even)."""
    b = np.ascontiguousarray(a, dtype=np.float32).view(np.uint32)
    r = (b + np.uint32(0x7FF) + ((b >> np.uint32(12)) & np.uint32(1))) & np.uint32(0xFFFFF000)
    return r.view(np.float32)


def _f16(a: np.ndarray):
    return np.ascontiguousarray(a).astype(np.float16)


def _perm128() -> np.ndarray:
    # evens then odds within one head's 128 dims
    return np.concatenate([np.arange(0, HD, 2), np.arange(1, HD, 2)])


def _rope_tables(t: int, norm_w: np.ndarray) -> tuple[np.ndarray, np.ndarray]:
    """A, B tables (128, t) for rope in permuted-QT layout, norm weight
    folded in: newQT = QT * A + SWAP64(QT) * B."""
    inv_freq = (1.0 / (ROPE_THETA ** (np.arange(0, HD, 2).astype(np.float32) / HD))).astype(np.float32)
    ang = np.arange(t, dtype=np.float32)[:, None] * inv_freq[None, :]  # (t, 64)
    cos = np.cos(ang).T.astype(np.float32)  # (64, t)
    sin = np.sin(ang).T.astype(np.float32)
    w = norm_w[_perm128()].astype(np.float32)  # (128,)
    a = np.concatenate([cos, cos], axis=0) * w[:, None]
    b = np.concatenate([-sin, sin], axis=0) * w[:, None]
    return np.ascontiguousarray(a), np.ascontiguousarray(b)


def _swap64() -> np.ndarray:
    # lhsT for out = SWAP64(rhs): lhsT[k, p] = 1 iff k == (p + 64) % 128
    p = np.arange(128)
    m = np.zeros((128, 128), dtype=np.float32)
    m[(p + 64) % 128, p] = 1.0
    return m


def _tri() -> np.ndarray:
    # scores^T diagonal-block mask: rows kk (key), cols qq (query), valid kk<=qq
    kk = np.arange(128)[:, None]
    qq = np.arange(128)[None, :]
    return np.where(kk <= qq, 0.0, NEG).astype(np.float32)


def _pack_w(w_rows: np.ndarray) -> np.ndarray:
    """(128, D) weight rows -> (128, KT*128) fp16, wt[p, 128k+i] = W[i, 128k+p]."""
    return _f16(w_rows.T.reshape(KT, 128, 128).transpose(1, 0, 2).reshape(128, KT * 128))


def _pack_x_chunk(xT: np.ndarray, ci: int) -> np.ndarray:
    """xT (D, T) slice -> (128, KT*512) fp16, xc[p, 512k+j] = xT[128k+p, 512ci+j]."""
    s = xT[:, 512 * ci : 512 * (ci + 1)]
    return _f16(s.reshape(KT, 128, 512).transpose(1, 0, 2).reshape(128, KT * 512))


# --------------------------------------------------------------------------
# device program
# --------------------------------------------------------------------------

def build_program(t: int):
    import concourse.bass as bass  # noqa: F401
    import concourse.tile as tile
    from concourse import bacc, mybir

    f32 = mybir.dt.float32
    f32r = mybir.dt.float32r
    f16 = mybir.dt.float16

    nch = t // 512

    nc = bacc.Bacc("TRN2", target_bir_lowering=False, debug=False, num_devices=N_CORES)

    io = {"t": t, "nch": nch}
    io["wtk_d"] = nc.dram_tensor("wtk", [128, KT * 128], f16, kind="ExternalInput").ap()
    for m in range(HPC):
        io[f"wtq{m}_d"] = nc.dram_tensor(f"wtq{m}", [128, KT * 128], f16, kind="ExternalInput").ap()
    io["wtv_d"] = nc.dram_tensor("wtv", [128, KT * 128], f16, kind="ExternalInput").ap()
    for ci in range(nch):
        io[f"xc{ci}_d"] = nc.dram_tensor(f"xc{ci}", [128, KT * 512], f16, kind="ExternalInput").ap()
    for h in range(HPC):
        io[f"wo{h}_d"] = nc.dram_tensor(f"wo{h}", [128, D], f16, kind="ExternalInput").ap()
    for nm in ("aq", "bq", "ak", "bk"):
        io[f"{nm}_d"] = nc.dram_tensor(nm, [128, t], f16, kind="ExternalInput").ap()
    io["p64_d"] = nc.dram_tensor("p64", [128, 128], f32r, kind="ExternalInput").ap()
    io["tri_d"] = nc.dram_tensor("tri", [128, 128], f32, kind="ExternalInput").ap()
    io["ones_d"] = nc.dram_tensor("ones", [128, 128], f16, kind="ExternalInput").ap()
    io["ident_d"] = nc.dram_tensor("ident", [128, 128], f16, kind="ExternalInput").ap()
    io["out_d"] = nc.dram_tensor("out_partial", [t // 128, 128, D], f16, kind="ExternalOutput").ap()

    with tile.TileContext(nc) as tc:
        _build_tile(tc, io)

    nc.compile()
    return nc


def _build_tile(tc, io):
    from concourse import mybir

    nc = tc.nc
    f32 = mybir.dt.float32
    f32r = mybir.dt.float32r
    f16 = mybir.dt.float16
    AF = mybir.ActivationFunctionType

    t = io["t"]
    nch = io["nch"]
    out_d = io["out_d"]

    with (
        tc.tile_pool(name="persist", bufs=1) as pp,
        tc.tile_pool(name="xcp", bufs=3) as xcp,
        tc.tile_pool(name="ps_proj", bufs=2, space="PSUM") as ps_proj,
        tc.tile_pool(name="ps_w", bufs=3, space="PSUM") as ps_w,
        tc.tile_pool(name="ps_av", bufs=2, space="PSUM") as ps_av,
        tc.tile_pool(name="ps_sum", bufs=1, space="PSUM") as ps_sum,
    ):
        # ---- persistent sbuf ----
        qtb = [pp.tile([128, t], f16, tag=f"qtb{h}", name=f"qtb{h}") for h in range(HPC)]
        ktb = pp.tile([128, t], f16, tag="ktb", name="ktb")
        vb = pp.tile([128, t], f16, tag="vb", name="vb")
        wt = {}
        wt["k"] = pp.tile([128, KT * 128], f16, tag="wtk", name="wtk")
        for m in range(HPC):
            wt[f"q{m}"] = pp.tile([128, KT * 128], f16, tag=f"wtq{m}", name=f"wtq{m}")
        wt["v"] = pp.tile([128, KT * 128], f16, tag="wtv", name="wtv")
        wo_t = [pp.tile([128, D], f16, tag=f"wo{h}", name=f"wo{h}") for h in range(HPC)]
        a_q = pp.tile([128, t], f16, tag="a_q", name="a_q")
        b_q = pp.tile([128, t], f16, tag="b_q", name="b_q")
        a_k = pp.tile([128, t], f16, tag="a_k", name="a_k")
        b_k = pp.tile([128, t], f16, tag="b_k", name="b_k")
        p64 = pp.tile([128, 128], f32r, tag="p64", name="p64")
        tri = pp.tile([128, 128], f32, tag="tri", name="tri")
        ones = pp.tile([128, 128], f16, tag="ones", name="ones")
        ident = pp.tile([128, 128], f16, tag="ident", name="ident")
        c_eps = pp.tile([128, 1], f32, tag="c_eps", name="c_eps")
        c_shift = pp.tile([128, 1], f32, tag="c_shift", name="c_shift")

        # ---- input DMA: single sync (HWDGE) queue = strict priority order.
        # The first projection chain needs wt_k + xc0 (2.5MB) before anything
        # else; everything later is ordered by first use.
        nc.sync.dma_start(wt["k"][:], io["wtk_d"])
        xc = {}
        for ci in range(3):
            xc[ci] = xcp.tile([128, KT * 512], f16, tag="xc", name=f"xc{ci}")
        nc.sync.dma_start(xc[0][:], io["xc0_d"])
        nc.sync.dma_start(wt["q0"][:], io["wtq0_d"])
        nc.sync.dma_start(p64[:], io["p64_d"])
        nc.sync.dma_start(tri[:], io["tri_d"])
        nc.sync.dma_start(ones[:], io["ones_d"])
        nc.sync.dma_start(ident[:], io["ident_d"])
        nc.sync.dma_start(a_k[:], io["ak_d"])
        nc.sync.dma_start(b_k[:], io["bk_d"])
        nc.sync.dma_start(a_q[:], io["aq_d"])
        nc.sync.dma_start(b_q[:], io["bq_d"])
        nc.sync.dma_start(wt["q1"][:], io["wtq1_d"])
        nc.sync.dma_start(wt["q2"][:], io["wtq2_d"])
        nc.sync.dma_start(wt["q3"][:], io["wtq3_d"])
        nc.sync.dma_start(wt["v"][:], io["wtv_d"])
        nc.sync.dma_start(xc[1][:], io["xc1_d"])
        for h in range(HPC):
            nc.sync.dma_start(wo_t[h][:], io[f"wo{h}_d"])
        nc.sync.dma_start(xc[2][:], io["xc2_d"])
        # xc3 is DMA'd after chunk 0's projections free the xc buffer slot

        nc.gpsimd.memset(c_eps[:], EPS)
        nc.gpsimd.memset(c_shift[:], ESHIFT)

        with (
            tc.tile_pool(name="rawp", bufs=3) as rawp,
            tc.tile_pool(name="sqp", bufs=2) as sqp,
            tc.tile_pool(name="scrp", bufs=3) as scrp,
            tc.tile_pool(name="lnp", bufs=6) as lnp,
            tc.tile_pool(name="q1p", bufs=6) as q1p,
            tc.tile_pool(name="rtp", bufs=2) as rtp,
            tc.tile_pool(name="vtp", bufs=2) as vtp,
            tc.tile_pool(name="gatep", bufs=2) as gatep,
            tc.tile_pool(name="exp", bufs=12) as expool,
            tc.tile_pool(name="rsp", bufs=4) as rspool,
            tc.tile_pool(name="obp", bufs=2) as obp,
        ):
            targets = ["k", "q0", "q1", "q2", "q3", "v"]

            def emit_wo_slice(ci, mi):
                m = 4 * ci + mi
                ob = obp.tile([128, D], f16, tag="ob", name="ob")
                for n in range(D // 512):
                    wops = ps_proj.tile([128, 512], f32, tag="proj", name="wo_ps")
                    for h in range(HPC):
                        nc.tensor.matmul(
                            wops[:],
                            yt[h][:, 128 * m : 128 * (m + 1)],
                            wo_t[h][:, 512 * n : 512 * (n + 1)],
                            start=(h == 0),
                            stop=(h == HPC - 1),
                        )
                    if (m + n) % 2 == 0:
                        nc.scalar.copy(ob[:, 512 * n : 512 * (n + 1)], wops[:])
                    else:
                        nc.vector.tensor_copy(ob[:, 512 * n : 512 * (n + 1)], wops[:])
                nc.sync.dma_start(out_d[m], ob[:])

            yt = [pp.tile([128, t], f16, tag=f"yt{h}", name=f"yt{h}") for h in range(HPC)]

            for ci in range(nch):
                sl = slice(512 * ci, 512 * (ci + 1))
                # ---- projections, rope-core one target late (the ssq matmul
                # trails an ACT copy + 1.5us gpsimd square; emitting it after
                # the NEXT target's 16 projection matmuls keeps it off the
                # tensor engine's critical path) ----
                lnts = {}
                q1bs = {}
                vt = None
                pending = None

                def emit_rope_core(tgt, raw, sq):
                    ssq = ps_w.tile([128, 512], f32, tag="w", name="ssq_ps")
                    nc.tensor.matmul(ssq[:], ones[:], sq[:])
                    lnt = lnp.tile([128, 512], f16, tag="lnt", name="lnt")
                    nc.scalar.activation(lnt[:], ssq[:], AF.Ln, bias=c_eps[:], scale=1.0 / HD)
                    lnts[tgt] = lnt
                    swp = ps_w.tile([128, 512], f32, tag="w", name="swp_ps")
                    nc.tensor.matmul(swp[:], p64[:], raw[:])
                    atab, btab = (a_k, b_k) if tgt == "k" else (a_q, b_q)
                    q1 = scrp.tile([128, 512], f32, tag="scr", name="q1")
                    nc.vector.tensor_mul(q1[:], raw[:], atab[:, sl])
                    m2 = scrp.tile([128, 512], f32, tag="scr", name="m2")
                    nc.vector.tensor_mul(m2[:], swp[:], btab[:, sl])
                    q1b = q1p.tile([128, 512], f16, tag="q1b", name="q1b")
                    nc.vector.tensor_add(q1b[:], q1[:], m2[:])
                    q1bs[tgt] = q1b

                for tgt in targets:
                    ps = ps_proj.tile([128, 512], f32, tag="proj", name="proj_ps")
                    for k in range(KT):
                        nc.tensor.matmul(
                            ps[:],
                            wt[tgt][:, 128 * k : 128 * (k + 1)],
                            xc[ci][:, 512 * k : 512 * (k + 1)],
                            start=(k == 0),
                            stop=(k == KT - 1),
                        )
                    if tgt == "v":
                        vt = vtp.tile([128, 512], f16, tag="vt", name="vt")
                        nc.vector.tensor_copy(vt[:], ps[:])
                    else:
                        raw = rawp.tile([128, 512], f32r, tag="raw", name="raw")
                        nc.scalar.copy(raw[:], ps[:])
                        sq = sqp.tile([128, 512], f16, tag="sq", name="sq")
                        nc.gpsimd.tensor_mul(sq[:], raw[:], raw[:])
                    if pending is not None:
                        emit_rope_core(*pending)
                    pending = (tgt, raw, sq) if tgt != "v" else None
                emit_rope_core("q3", raw, sq)

                # xc3 load goes in the slot xc0 frees up
                if ci == 0 and nch > 3:
                    xc[3] = xcp.tile([128, KT * 512], f16, tag="xc", name="xc3")
                    nc.sync.dma_start(xc[3][:], io["xc3_d"])

                # ---- V transpose: (hd, Tk) -> (Tk, hd) blocks ----
                for c4 in range(4):
                    vps = ps_w.tile([128, 128], f16, tag="w", name="vtr_ps")
                    nc.tensor.transpose(vps[:], vt[:, 128 * c4 : 128 * (c4 + 1)], ident[:])
                    nc.vector.tensor_copy(vb[:, 512 * ci + 128 * c4 : 512 * ci + 128 * (c4 + 1)], vps[:])

                # ---- finals: r = exp(-0.5*ln(mean+eps)) [+ head-dim scale
                # for k]. Biases route through gate tiles derived from the
                # LAST Ln so the list scheduler cannot hoist any Exp between
                # the Lns (each Ln<->Exp flip costs a 1.28us ACT table load).
                gate_z = gatep.tile([128, 1], f32, tag="gate", name="gate_z")
                gate_k = gatep.tile([128, 1], f32, tag="gate", name="gate_k")
                nc.vector.tensor_scalar_mul(gate_z[:], lnts["q3"][:, 0:1], 0.0)
                nc.vector.tensor_scalar_add(gate_k[:], gate_z[:], -0.5 * math.log(float(HD)))
                for tgt in ["k", "q0", "q1", "q2", "q3"]:
                    dstb = ktb if tgt == "k" else qtb[int(tgt[1])]
                    bias = gate_k if tgt == "k" else gate_z
                    r_t = rtp.tile([128, 512], f32, tag="r_t", name="r_t")
                    nc.scalar.activation(r_t[:], lnts[tgt][:], AF.Exp, bias=bias[:], scale=-0.5)
                    nc.vector.tensor_mul(dstb[:, sl], q1bs[tgt][:], r_t[:])

                # ---- attention for chunk ci; wo m-slices of the previous
                # chunk (inputs long since ready) interleave after each head ----
                for h in range(HPC):
                    av = ps_av.tile([128, 512], f32, tag="av", name="av_ps")
                    sums = ps_sum.tile([128, 512], f32, tag="sums", name="sums_ps")
                    nb = 4 * ci + 4
                    for c in range(nb):
                        diag = c >= 4 * ci
                        r = c - 4 * ci if diag else 0
                        w0 = 128 * r
                        sc = ps_w.tile([128, 512], f32, tag="w", name="sc_ps")
                        nc.tensor.matmul(
                            sc[:, w0:512],
                            ktb[:, 128 * c : 128 * (c + 1)],
                            qtb[h][:, 512 * ci + w0 : 512 * (ci + 1)],
                        )
                        if diag:
                            nc.vector.tensor_add(sc[:, w0 : w0 + 128], sc[:, w0 : w0 + 128], tri[:])
                        ex = expool.tile([128, 512], f16, tag="ex", name="ex")
                        nc.scalar.activation(ex[:, w0:512], sc[:, w0:512], AF.Exp, bias=c_shift[:])
                        nc.tensor.matmul(
                            sums[:, w0:512],
                            ones[:],
                            ex[:, w0:512],
                            start=(c == 0),
                            stop=(c == nb - 1),
                        )
                        nc.tensor.matmul(
                            av[:, w0:512],
                            vb[:, 128 * c : 128 * (c + 1)],
                            ex[:, w0:512],
                            start=(c == 0),
                            stop=(c == nb - 1),
                        )
                    rs = rspool.tile([128, 512], f32, tag="rs", name="rs")
                    rs2 = rspool.tile([128, 512], f32, tag="rs", name="rs2")
                    nc.vector.reciprocal_approx_accurate(rs[:], sums[:], rs2[:])
                    nc.vector.tensor_mul(yt[h][:, sl], av[:], rs[:])
                    if ci > 0:
                        emit_wo_slice(ci - 1, h)

            for mi in range(4):
                emit_wo_slice(nch - 1, mi)


# --------------------------------------------------------------------------
# host wrapper
# --------------------------------------------------------------------------

_PROGRAM_CACHE: dict[int, object] = {}
TRACE = False


def _get_program(t: int):
    if t not in _PROGRAM_CACHE:
        _PROGRAM_CACHE[t] = build_program(t)
    return _PROGRAM_CACHE[t]


def make_core_inputs(x, wq, wk, wv, wo, q_norm_w, k_norm_w, t: int):
    """Build the 8 per-core input dicts (numpy, host-side sharding)."""
    perm = _perm128()
    aq, bq = _rope_tables(t, q_norm_w)
    ak, bk = _rope_tables(t, k_norm_w)
    aq, bq, ak, bk = (v.astype(np.float16) for v in (aq, bq, ak, bk))
    p64 = round_fp32r(_swap64())
    tri = _tri()
    ones = np.ones((128, 128), dtype=np.float16)
    ident = np.eye(128, dtype=np.float32).astype(np.float16)

    nch = t // 512
    xcs = {}  # (b, ci) -> packed chunk
    for b in range(B):
        xT = np.ascontiguousarray(x[b].T)
        for ci in range(nch):
            xcs[(b, ci)] = _pack_x_chunk(xT, ci)

    in_maps = []
    for core in range(N_CORES):
        b = core // N_KV_HEAD
        j = core % N_KV_HEAD
        m = {}
        krows = 128 * j + perm
        m["wtk"] = _pack_w(wk[krows, :])
        for hh in range(HPC):
            qrows = 128 * (HPC * j + hh) + perm
            m[f"wtq{hh}"] = _pack_w(wq[qrows, :])
        m["wtv"] = _pack_w(wv[128 * j : 128 * (j + 1), :])
        for ci in range(nch):
            m[f"xc{ci}"] = xcs[(b, ci)]
        for h in range(HPC):
            m[f"wo{h}"] = _f16(wo[:, 512 * j + 128 * h : 512 * j + 128 * (h + 1)].T)
        m.update({"aq": aq, "bq": bq, "ak": ak, "bk": bk, "p64": p64, "tri": tri,
                  "ones": ones, "ident": ident})
        in_maps.append(m)
    return in_maps


def kernel(x, wq, wk, wv, wo, q_norm_w, k_norm_w):
    x = np.asarray(x, dtype=np.float32)
    wq = np.asarray(wq, dtype=np.float32)
    wk = np.asarray(wk, dtype=np.float32)
    wv = np.asarray(wv, dtype=np.float32)
    wo = np.asarray(wo, dtype=np.float32)
    q_norm_w = np.asarray(q_norm_w, dtype=np.float32)
    k_norm_w = np.asarray(k_norm_w, dtype=np.float32)

    t = x.shape[1]
    nc = _get_program(t)
    in_maps = make_core_inputs(x, wq, wk, wv, wo, q_norm_w, k_norm_w, t)

    from concourse import bass_utils

    res = bass_utils.run_bass_kernel_spmd(
        nc,
        in_maps,
        core_ids=list(range(N_CORES)),
        trace=TRACE,
        trace_cores=[0] if TRACE else None,
    )
    kernel.last_results = res

    out = np.zeros((B, t, D), dtype=np.float32)
    for core in range(N_CORES):
        b = core // N_KV_HEAD
        out[b] += res.results[core]["out_partial"].reshape(t, D).astype(np.float32)
    return out


kernel.last_results = None


# revision 15
# speedup vs baseline: 1.1054x; 1.0244x over previous
"""Causal self-attention (GQA + RoPE + QK-RMSNorm) Trainium2 Bass kernel.

Sharding (8 cores): core c -> batch b = c//4, kv-head j = c%4, q-heads 4j..4j+3.
Each core computes its 4 heads' attention for its batch plus the partial
output projection against wo[:, 512j:512j+512]; the host sums the 4 partials
per batch.

v2: single chunk-pipelined pass. Per 512-token chunk ci:
  proj(K,Q0..Q3,V over xc[ci]) -> rope-core + rms-Ln per target ->
  V-transpose -> finals (Exp+mul -> ktb/qtb chunk) -> wo(ci-1) -> attention(ci).
This keeps the tensor engine fed across phase boundaries: the next chunk's
projections fill attention's exp/recip dependency bubbles, and wo is emitted
one chunk late so its yt inputs are long since ready.

All host-side tensors are packed so every DMA is (128, N) contiguous per
partition (4-16KB descriptors). fp16 operands everywhere (same PE rate as
bf16, 8x less quantization noise); attention exp is exp(s-2) to keep the
row sums comfortably inside fp16 range (the 2 cancels in the softmax ratio).
"""

import math

import numpy as np

B, T, D = 2, 2048, 2048
N_HEAD, N_KV_HEAD = 16, 4
HD = 128
HPC = N_HEAD // N_KV_HEAD  # q heads per core = 4
N_CORES = 8
ROPE_THETA = 10000.0
EPS = float(np.finfo(np.float32).eps)
NEG = -1.0e5
ESHIFT = -2.0  # attention exp computes exp(s + ESHIFT); cancels in av/sums

KT = D // 128  # 16 contraction k-tiles
NCH = T // 512  # 4 Tq chunks


# --------------------------------------------------------------------------
# host-side constant tables
# --------------------------------------------------------------------------

def round_fp32r(a: np.ndarray) -> np.ndarray:
    """Round fp32 to the fp32r grid (11-bit mantissa, round-to-nearest-even)."""
    b = np.ascontiguousarray(a, dtype=np.float32).view(np.uint32)
    r = (b + np.uint32(0x7FF) + ((b >> np.uint32(12)) & np.uint32(1))) & np.uint32(0xFFFFF000)
    return r.view(np.float32)


def _f16(a: np.ndarray):
    return np.ascontiguousarray(a).astype(np.float16)


def _perm128() -> np.ndarray:
    # evens then odds within one head's 128 dims
    return np.concatenate([np.arange(0, HD, 2), np.arange(1, HD, 2)])


def _rope_tables(t: int, norm_w: np.ndarray) -> tuple[np.ndarray, np.ndarray]:
    """A, B tables (128, t) for rope in permuted-QT layout, norm weight
    folded in: newQT = QT * A + SWAP64(QT) * B."""
    inv_freq = (1.0 / (ROPE_THETA ** (np.arange(0, HD, 2).astype(np.float32) / HD))).astype(np.float32)
    ang = np.arange(t, dtype=np.float32)[:, None] * inv_freq[None, :]  # (t, 64)
    cos = np.cos(ang).T.astype(np.float32)  # (64, t)
    sin = np.sin(ang).T.astype(np.float32)
    w = norm_w[_perm128()].astype(np.float32)  # (128,)
    a = np.concatenate([cos, cos], axis=0) * w[:, None]
    b = np.concatenate([-sin, sin], axis=0) * w[:, None]
    return np.ascontiguousarray(a), np.ascontiguousarray(b)


def _swap64() -> np.ndarray:
    # lhsT for out = SWAP64(rhs): lhsT[k, p] = 1 iff k == (p + 64) % 128
    p = np.arange(128)
    m = np.zeros((128, 128), dtype=np.float32)
    m[(p + 64) % 128, p] = 1.0
    return m


def _tri() -> np.ndarray:
    # scores^T diagonal-block mask: rows kk (key), cols qq (query), valid kk<=qq
    kk = np.arange(128)[:, None]
    qq = np.arange(128)[None, :]
    return np.where(kk <= qq, 0.0, NEG).astype(np.float32)


def _pack_w(w_rows: np.ndarray) -> np.ndarray:
    """(128, D) weight rows -> (128, KT*128) fp16, wt[p, 128k+i] = W[i, 128k+p]."""
    return _f16(w_rows.T.reshape(KT, 128, 128).transpose(1, 0, 2).reshape(128, KT * 128))


def _pack_x_chunk(xT: np.ndarray, ci: int) -> np.ndarray:
    """xT (D, T) slice -> (128, KT*512) fp16, xc[p, 512k+j] = xT[128k+p, 512ci+j]."""
    s = xT[:, 512 * ci : 512 * (ci + 1)]
    return _f16(s.reshape(KT, 128, 512).transpose(1, 0, 2).reshape(128, KT * 512))


# --------------------------------------------------------------------------
# device program
# --------------------------------------------------------------------------

def build_program(t: int):
    import concourse.bass as bass  # noqa: F401
    import concourse.tile as tile
    from concourse import bacc, mybir

    f32 = mybir.dt.float32
    f32r = mybir.dt.float32r
    f16 = mybir.dt.float16

    nch = t // 512

    nc = bacc.Bacc("TRN2", target_bir_lowering=False, debug=False, num_devices=N_CORES)

    io = {"t": t, "nch": nch}
    io["wtk_d"] = nc.dram_tensor("wtk", [128, KT * 128], f16, kind="ExternalInput").ap()
    for m in range(HPC):
        io[f"wtq{m}_d"] = nc.dram_tensor(f"wtq{m}", [128, KT * 128], f16, kind="ExternalInput").ap()
    io["wtv_d"] = nc.dram_tensor("wtv", [128, KT * 128], f16, kind="ExternalInput").ap()
    for ci in range(nch):
        io[f"xc{ci}_d"] = nc.dram_tensor(f"xc{ci}", [128, KT * 512], f16, kind="ExternalInput").ap()
    for h in range(HPC):
        io[f"wo{h}_d"] = nc.dram_tensor(f"wo{h}", [128, D], f16, kind="ExternalInput").ap()
    for nm in ("aq", "bq", "ak", "bk"):
        io[f"{nm}_d"] = nc.dram_tensor(nm, [128, t], f16, kind="ExternalInput").ap()
    io["p64_d"] = nc.dram_tensor("p64", [128, 128], f32r, kind="ExternalInput").ap()
    io["tri_d"] = nc.dram_tensor("tri", [128, 128], f32, kind="ExternalInput").ap()
    io["ones_d"] = nc.dram_tensor("ones", [128, 128], f16, kind="ExternalInput").ap()
    io["ident_d"] = nc.dram_tensor("ident", [128, 128], f16, kind="ExternalInput").ap()
    io["out_d"] = nc.dram_tensor("out_partial", [t // 128, 128, D], f16, kind="ExternalOutput").ap()

    with tile.TileContext(nc) as tc:
        _build_tile(tc, io)

    nc.compile()
    return nc


def _build_tile(tc, io):
    from concourse import mybir

    nc = tc.nc
    f32 = mybir.dt.float32
    f32r = mybir.dt.float32r
    f16 = mybir.dt.float16
    AF = mybir.ActivationFunctionType

    t = io["t"]
    nch = io["nch"]
    out_d = io["out_d"]

    with (
        tc.tile_pool(name="persist", bufs=1) as pp,
        tc.tile_pool(name="xcp", bufs=3) as xcp,
        tc.tile_pool(name="ps_proj", bufs=2, space="PSUM") as ps_proj,
        tc.tile_pool(name="ps_w", bufs=3, space="PSUM") as ps_w,
        tc.tile_pool(name="ps_av", bufs=2, space="PSUM") as ps_av,
        tc.tile_pool(name="ps_sum", bufs=1, space="PSUM") as ps_sum,
    ):
        # ---- persistent sbuf ----
        qtb = [pp.tile([128, t], f16, tag=f"qtb{h}", name=f"qtb{h}") for h in range(HPC)]
        ktb = pp.tile([128, t], f16, tag="ktb", name="ktb")
        vb = pp.tile([128, t], f16, tag="vb", name="vb")
        wt = {}
        wt["k"] = pp.tile([128, KT * 128], f16, tag="wtk", name="wtk")
        for m in range(HPC):
            wt[f"q{m}"] = pp.tile([128, KT * 128], f16, tag=f"wtq{m}", name=f"wtq{m}")
        wt["v"] = pp.tile([128, KT * 128], f16, tag="wtv", name="wtv")
        wo_t = [pp.tile([128, D], f16, tag=f"wo{h}", name=f"wo{h}") for h in range(HPC)]
        a_q = pp.tile([128, t], f16, tag="a_q", name="a_q")
        b_q = pp.tile([128, t], f16, tag="b_q", name="b_q")
        a_k = pp.tile([128, t], f16, tag="a_k", name="a_k")
        b_k = pp.tile([128, t], f16, tag="b_k", name="b_k")
        p64 = pp.tile([128, 128], f32r, tag="p64", name="p64")
        tri = pp.tile([128, 128], f32, tag="tri", name="tri")
        ones = pp.tile([128, 128], f16, tag="ones", name="ones")
        ident = pp.tile([128, 128], f16, tag="ident", name="ident")
        c_eps = pp.tile([128, 1], f32, tag="c_eps", name="c_eps")
        c_shift = pp.tile([128, 1], f32, tag="c_shift", name="c_shift")

        # ---- input DMA: single sync (HWDGE) queue = strict priority order.
        # The first projection chain needs wt_k + xc0 (2.5MB) before anything
        # else; everything later is ordered by first use.
        nc.sync.dma_start(wt["k"][:], io["wtk_d"])
        xc = {}
        for ci in range(3):
            xc[ci] = xcp.tile([128, KT * 512], f16, tag="xc", name=f"xc{ci}")
        nc.sync.dma_start(xc[0][:], io["xc0_d"])
        nc.sync.dma_start(wt["q0"][:], io["wtq0_d"])
        nc.sync.dma_start(p64[:], io["p64_d"])
        nc.sync.dma_start(tri[:], io["tri_d"])
        nc.sync.dma_start(ones[:], io["ones_d"])
        nc.sync.dma_start(ident[:], io["ident_d"])
        nc.sync.dma_start(a_k[:], io["ak_d"])
        nc.sync.dma_start(b_k[:], io["bk_d"])
        nc.sync.dma_start(a_q[:], io["aq_d"])
        nc.sync.dma_start(b_q[:], io["bq_d"])
        nc.sync.dma_start(wt["q1"][:], io["wtq1_d"])
        nc.sync.dma_start(wt["q2"][:], io["wtq2_d"])
        nc.sync.dma_start(wt["q3"][:], io["wtq3_d"])
        nc.sync.dma_start(wt["v"][:], io["wtv_d"])
        nc.sync.dma_start(xc[1][:], io["xc1_d"])
        for h in range(HPC):
            nc.sync.dma_start(wo_t[h][:], io[f"wo{h}_d"])
        nc.sync.dma_start(xc[2][:], io["xc2_d"])
        # xc3 is DMA'd after chunk 0's projections free the xc buffer slot

        nc.gpsimd.memset(c_eps[:], EPS)
        nc.gpsimd.memset(c_shift[:], ESHIFT)

        with (
            tc.tile_pool(name="rawp", bufs=3) as rawp,
            tc.tile_pool(name="sqp", bufs=2) as sqp,
            tc.tile_pool(name="scrp", bufs=3) as scrp,
            tc.tile_pool(name="lnp", bufs=6) as lnp,
            tc.tile_pool(name="q1p", bufs=6) as q1p,
            tc.tile_pool(name="rtp", bufs=2) as rtp,
            tc.tile_pool(name="vtp", bufs=2) as vtp,
            tc.tile_pool(name="gatep", bufs=2) as gatep,
            tc.tile_pool(name="exp", bufs=12) as expool,
            tc.tile_pool(name="rsp", bufs=4) as rspool,
            tc.tile_pool(name="obp", bufs=2) as obp,
        ):
            targets = ["k", "q0", "q1", "q2", "q3", "v"]

            def emit_wo_slice(ci, mi):
                m = 4 * ci + mi
                ob = obp.tile([128, D], f16, tag="ob", name="ob")
                for n in range(D // 512):
                    wops = ps_proj.tile([128, 512], f32, tag="proj", name="wo_ps")
                    for h in range(HPC):
                        nc.tensor.matmul(
                            wops[:],
                            yt[h][:, 128 * m : 128 * (m + 1)],
                            wo_t[h][:, 512 * n : 512 * (n + 1)],
                            start=(h == 0),
                            stop=(h == HPC - 1),
                        )
                    if (m + n) % 2 == 0:
                        nc.scalar.copy(ob[:, 512 * n : 512 * (n + 1)], wops[:])
                    else:
                        nc.vector.tensor_copy(ob[:, 512 * n : 512 * (n + 1)], wops[:])
                nc.sync.dma_start(out_d[m], ob[:])

            yt = [pp.tile([128, t], f16, tag=f"yt{h}", name=f"yt{h}") for h in range(HPC)]

            for ci in range(nch):
                sl = slice(512 * ci, 512 * (ci + 1))
                # ---- projections, rope-core one target late (the ssq matmul
                # trails an ACT copy + 1.5us gpsimd square; emitting it after
                # the NEXT target's 16 projection matmuls keeps it off the
                # tensor engine's critical path) ----
                lnts = {}
                q1bs = {}
                vt = None
                pending = None

                def emit_rope_core(tgt, raw, sq):
                    ssq = ps_w.tile([128, 512], f32, tag="w", name="ssq_ps")
                    nc.tensor.matmul(ssq[:], ones[:], sq[:])
                    lnt = lnp.tile([128, 512], f16, tag="lnt", name="lnt")
                    nc.scalar.activation(lnt[:], ssq[:], AF.Ln, bias=c_eps[:], scale=1.0 / HD)
                    lnts[tgt] = lnt
                    swp = ps_w.tile([128, 512], f32, tag="w", name="swp_ps")
                    nc.tensor.matmul(swp[:], p64[:], raw[:])
                    atab, btab = (a_k, b_k) if tgt == "k" else (a_q, b_q)
                    q1 = scrp.tile([128, 512], f32, tag="scr", name="q1")
                    nc.vector.tensor_mul(q1[:], raw[:], atab[:, sl])
                    m2 = scrp.tile([128, 512], f32, tag="scr", name="m2")
                    nc.vector.tensor_mul(m2[:], swp[:], btab[:, sl])
                    q1b = q1p.tile([128, 512], f16, tag="q1b", name="q1b")
                    nc.vector.tensor_add(q1b[:], q1[:], m2[:])
                    q1bs[tgt] = q1b

                for tgt in targets:
                    ps = ps_proj.tile([128, 512], f32, tag="proj", name="proj_ps")
                    for k in range(KT):
                        nc.tensor.matmul(
                            ps[:],
                            wt[tgt][:, 128 * k : 128 * (k + 1)],
                            xc[ci][:, 512 * k : 512 * (k + 1)],
                            start=(k == 0),
                            stop=(k == KT - 1),
                        )
                    if tgt == "v":
                        vt = vtp.tile([128, 512], f16, tag="vt", name="vt")
                        nc.vector.tensor_copy(vt[:], ps[:])
                    else:
                        raw = rawp.tile([128, 512], f32r, tag="raw", name="raw")
                        nc.scalar.copy(raw[:], ps[:])
                        sq = sqp.tile([128, 512], f16, tag="sq", name="sq")
                        nc.gpsimd.tensor_mul(sq[:], raw[:], raw[:])
                    if pending is not None:
                        emit_rope_core(*pending)
                    pending = (tgt, raw, sq) if tgt != "v" else None

                # xc3 load goes in the slot xc0 frees up
                if ci == 0 and nch > 3:
                    xc[3] = xcp.tile([128, KT * 512], f16, tag="xc", name="xc3")
                    nc.sync.dma_start(xc[3][:], io["xc3_d"])

                # ---- V transpose: (hd, Tk) -> (Tk, hd) blocks ----
                for c4 in range(4):
                    vps = ps_w.tile([128, 128], f16, tag="w", name="vtr_ps")
                    nc.tensor.transpose(vps[:], vt[:, 128 * c4 : 128 * (c4 + 1)], ident[:])
                    nc.vector.tensor_copy(vb[:, 512 * ci + 128 * c4 : 512 * ci + 128 * (c4 + 1)], vps[:])

                # ---- finals: r = exp(-0.5*ln(mean+eps)) [+ head-dim scale
                # for k]. Biases route through gate tiles derived from the
                # LAST Ln so the list scheduler cannot hoist any Exp between
                # the Lns (each Ln<->Exp flip costs a 1.28us ACT table load).
                gate_z = gatep.tile([128, 1], f32, tag="gate", name="gate_z")
                gate_k = gatep.tile([128, 1], f32, tag="gate", name="gate_k")
                nc.vector.tensor_scalar_mul(gate_z[:], lnts["q3"][:, 0:1], 0.0)
                nc.vector.tensor_scalar_add(gate_k[:], gate_z[:], -0.5 * math.log(float(HD)))
                for tgt in ["k", "q0", "q1", "q2", "q3"]:
                    dstb = ktb if tgt == "k" else qtb[int(tgt[1])]
                    bias = gate_k if tgt == "k" else gate_z
                    r_t = rtp.tile([128, 512], f32, tag="r_t", name="r_t")
                    nc.scalar.activation(r_t[:], lnts[tgt][:], AF.Exp, bias=bias[:], scale=-0.5)
                    nc.vector.tensor_mul(dstb[:, sl], q1bs[tgt][:], r_t[:])

                # ---- attention for chunk ci; wo m-slices of the previous
                # chunk (inputs long since ready) interleave after each head ----
                for h in range(HPC):
                    av = ps_av.tile([128, 512], f32, tag="av", name="av_ps")
                    sums = ps_sum.tile([128, 512], f32, tag="sums", name="sums_ps")
                    nb = 4 * ci + 4
                    for c in range(nb):
                        diag = c >= 4 * ci
                        r = c - 4 * ci if diag else 0
                        w0 = 128 * r
                        sc = ps_w.tile([128, 512], f32, tag="w", name="sc_ps")
                        nc.tensor.matmul(
                            sc[:, w0:512],
                            ktb[:, 128 * c : 128 * (c + 1)],
                            qtb[h][:, 512 * ci + w0 : 512 * (ci + 1)],
                        )
                        if diag:
                            nc.vector.tensor_add(sc[:, w0 : w0 + 128], sc[:, w0 : w0 + 128], tri[:])
                        ex = expool.tile([128, 512], f16, tag="ex", name="ex")
                        nc.scalar.activation(ex[:, w0:512], sc[:, w0:512], AF.Exp, bias=c_shift[:])
                        nc.tensor.matmul(
                            sums[:, w0:512],
                            ones[:],
                            ex[:, w0:512],
                            start=(c == 0),
                            stop=(c == nb - 1),
                        )
                        nc.tensor.matmul(
                            av[:, w0:512],
                            vb[:, 128 * c : 128 * (c + 1)],
                            ex[:, w0:512],
                            start=(c == 0),
                            stop=(c == nb - 1),
                        )
                    rs = rspool.tile([128, 512], f32, tag="rs", name="rs")
                    rs2 = rspool.tile([128, 512], f32, tag="rs", name="rs2")
                    nc.vector.reciprocal_approx_accurate(rs[:], sums[:], rs2[:])
                    nc.vector.tensor_mul(yt[h][:, sl], av[:], rs[:])
                    if ci > 0:
                        emit_wo_slice(ci - 1, h)

            for mi in range(4):
                emit_wo_slice(nch - 1, mi)


# --------------------------------------------------------------------------
# host wrapper
# --------------------------------------------------------------------------

_PROGRAM_CACHE: dict[int, object] = {}
TRACE = False


def _get_program(t: int):
    if t not in _PROGRAM_CACHE:
        _PROGRAM_CACHE[t] = build_program(t)
    return _PROGRAM_CACHE[t]


def make_core_inputs(x, wq, wk, wv, wo, q_norm_w, k_norm_w, t: int):
    """Build the 8 per-core input dicts (numpy, host-side sharding)."""
    perm = _perm128()
    aq, bq = _rope_tables(t, q_norm_w)
    ak, bk = _rope_tables(t, k_norm_w)
    aq, bq, ak, bk = (v.astype(np.float16) for v in (aq, bq, ak, bk))
    p64 = round_fp32r(_swap64())
    tri = _tri()
    ones = np.ones((128, 128), dtype=np.float16)
    ident = np.eye(128, dtype=np.float32).astype(np.float16)

    nch = t // 512
    xcs = {}  # (b, ci) -> packed chunk
    for b in range(B):
        xT = np.ascontiguousarray(x[b].T)
        for ci in range(nch):
            xcs[(b, ci)] = _pack_x_chunk(xT, ci)

    in_maps = []
    for core in range(N_CORES):
        b = core // N_KV_HEAD
        j = core % N_KV_HEAD
        m = {}
        krows = 128 * j + perm
        m["wtk"] = _pack_w(wk[krows, :])
        for hh in range(HPC):
            qrows = 128 * (HPC * j + hh) + perm
            m[f"wtq{hh}"] = _pack_w(wq[qrows, :])
        m["wtv"] = _pack_w(wv[128 * j : 128 * (j + 1), :])
        for ci in range(nch):
            m[f"xc{ci}"] = xcs[(b, ci)]
        for h in range(HPC):
            m[f"wo{h}"] = _f16(wo[:, 512 * j + 128 * h : 512 * j + 128 * (h + 1)].T)
        m.update({"aq": aq, "bq": bq, "ak": ak, "bk": bk, "p64": p64, "tri": tri,
                  "ones": ones, "ident": ident})
        in_maps.append(m)
    return in_maps


def kernel(x, wq, wk, wv, wo, q_norm_w, k_norm_w):
    x = np.asarray(x, dtype=np.float32)
    wq = np.asarray(wq, dtype=np.float32)
    wk = np.asarray(wk, dtype=np.float32)
    wv = np.asarray(wv, dtype=np.float32)
    wo = np.asarray(wo, dtype=np.float32)
    q_norm_w = np.asarray(q_norm_w, dtype=np.float32)
    k_norm_w = np.asarray(k_norm_w, dtype=np.float32)

    t = x.shape[1]
    nc = _get_program(t)
    in_maps = make_core_inputs(x, wq, wk, wv, wo, q_norm_w, k_norm_w, t)

    from concourse import bass_utils

    res = bass_utils.run_bass_kernel_spmd(
        nc,
        in_maps,
        core_ids=list(range(N_CORES)),
        trace=TRACE,
        trace_cores=[0] if TRACE else None,
    )
    kernel.last_results = res

    out = np.zeros((B, t, D), dtype=np.float32)
    for core in range(N_CORES):
        b = core // N_KV_HEAD
        out[b] += res.results[core]["out_partial"].reshape(t, D).astype(np.float32)
    return out


kernel.last_results = None
